# revision 7
# baseline (speedup 1.0000x reference)
# DropConnect LSTM cell kernel for Trainium2 (Bass/Tile), data-parallel over
# batch across 8 NeuronCores.
#
# Math (per reference):
#   x_d = x * (dp_u >= 0.1) / 0.9
#   h_d = h * (rec_dp_u >= 0.1) / 0.9
#   w   = kernel * (k_dp_u >= 0.05) / 0.95
#   rw  = recurrent_kernel * (rk_dp_u >= 0.05) / 0.95
#   z   = x_d @ w + h_d @ rw + bias          (split into gates i,f,c~,o)
#   c'  = sig(zf)*c + sig(zi)*tanh(zc)
#   h'  = sig(zo)*tanh(c')
#
# Kernel strategy (per core, B_c = 1024 batch rows):
#  - Matmul operands are fp16 (10 mantissa bits -> relerr ~1.3e-3, well under
#    the 2e-2 gate; fp16/bf16 matmuls also stream measurably faster than
#    fp32r on HW). The value tensors x/h/kernel/recurrent_kernel are staged
#    to fp16 on the host: (u>=rate)*fp16(v) == fp16((u>=rate)*v) exactly
#    (mask is 0/1), so results are bit-identical to on-device rounding while
#    halving those tensors' DMA bytes. The dropout-uniform tensors stay f32
#    because the comparisons (u >= rate) must be exact in f32.
#  - DMA instruction count is minimized (HWDGE issue costs ~625ns of shared
#    serialized time per DMA): activations load in [128,2,1024] blocks,
#    weights in [128,4,512] blocks, bias once, and h'/c' store in
#    [128,4,512] blocks straight out of the resident T_all buffer.
#  - Masks applied with one fused DVE scalar_tensor_tensor:
#    out = (u >= rate) * v, cast to fp16 on write.
#  - Activations are transposed on-chip with PE transposes (fp16 identity,
#    1.0 c/row vs 2.0 for f32) into per-(k,b) actT[128,128] fp16 tiles.
#  - Both 1/(1-rate) scales are identical for the x and h paths, so the
#    combined scale S = 1/(0.9*0.95) is applied once inside the gate
#    activations (out = f(S*psum)), and bias is pre-divided by S and
#    injected into PSUM with a K=1 matmul so psum = act_m@w_m + bias/S.
#  - Gate-ordered chunk loop (c~ -> i -> f -> o) so i*tanh(zc) and c' can be
#    accumulated in a single resident buffer.

from contextlib import ExitStack

import numpy as np

import concourse.bass as bass
import concourse.mybir as mybir
import concourse.tile as tile
from concourse import bacc
from concourse.bass_utils import run_bass_kernel_spmd
from concourse.masks import make_identity

N_CORES = 8
B, D, U = 8192, 1024, 1024
BC = B // N_CORES  # per-core batch rows
P = 128
NG4 = 4 * U  # 4096 gate columns
KT = (D + U) // P  # 16 contraction tiles
NW = 512  # matmul free-dim chunk

DROPOUT = 0.1
KERNEL_DROPOUT = 0.05
S = 1.0 / ((1.0 - DROPOUT) * (1.0 - KERNEL_DROPOUT))

f32 = mybir.dt.float32
f16 = mybir.dt.float16
AF = mybir.ActivationFunctionType
OP = mybir.AluOpType


def build_nc(bc: int = BC, repeat: int = 1):
    """Build and compile the per-core Bass program for per-core batch bc.

    repeat > 1 re-emits the whole computation N times in one NEFF (same
    inputs/outputs) — used only for device-time measurement in test.py.
    """
    btl = bc // P
    nc = bacc.Bacc("TRN2", target_bir_lowering=False, debug=False)

    x = nc.dram_tensor("x", [bc, D], f16, kind="ExternalInput").ap()
    h = nc.dram_tensor("h", [bc, U], f16, kind="ExternalInput").ap()
    c_in = nc.dram_tensor("c", [bc, U], f32, kind="ExternalInput").ap()
    dp = nc.dram_tensor("dp_u", [bc, D], f32, kind="ExternalInput").ap()
    rdp = nc.dram_tensor("rec_dp_u", [bc, U], f32, kind="ExternalInput").ap()
    kw = nc.dram_tensor("kern", [D, NG4], f16, kind="ExternalInput").ap()
    rkw = nc.dram_tensor("rkern", [U, NG4], f16, kind="ExternalInput").ap()
    kdp = nc.dram_tensor("k_dp_u", [D, NG4], f32, kind="ExternalInput").ap()
    rkdp = nc.dram_tensor("rk_dp_u", [U, NG4], f32, kind="ExternalInput").ap()
    bias = nc.dram_tensor("bias", [NG4], f32, kind="ExternalInput").ap()
    h_new = nc.dram_tensor("h_new", [bc, U], f32, kind="ExternalOutput").ap()
    c_new = nc.dram_tensor("c_new", [bc, U], f32, kind="ExternalOutput").ap()

    with tile.TileContext(nc) as tc, ExitStack() as ctx:
        const = ctx.enter_context(tc.tile_pool(name="const", bufs=1))
        xstage = ctx.enter_context(tc.tile_pool(name="xstage", bufs=2))
        ustage = ctx.enter_context(tc.tile_pool(name="ustage", bufs=2))
        vstage = ctx.enter_context(tc.tile_pool(name="vstage", bufs=2))
        atrans = ctx.enter_context(tc.tile_pool(name="atrans", bufs=1))
        w16s = ctx.enter_context(tc.tile_pool(name="w16s", bufs=4))
        wus = ctx.enter_context(tc.tile_pool(name="wus", bufs=4))
        # 24 weight-mask bufs = 1.5 chunks resident so chunk ci+1's masks
        # prefetch while ci's matmuls run
        wmpool = ctx.enter_context(tc.tile_pool(name="wm", bufs=24))
        gstage = ctx.enter_context(tc.tile_pool(name="gstage", bufs=5))
        cpool = ctx.enter_context(tc.tile_pool(name="cpool", bufs=6))
        bstage = ctx.enter_context(tc.tile_pool(name="bstage", bufs=1))
        tpool = ctx.enter_context(tc.tile_pool(name="tpool", bufs=1))
        psum = ctx.enter_context(tc.tile_pool(name="psum", bufs=8, space="PSUM"))

        identr = const.tile([P, P], f32)
        make_identity(nc, identr)
        ident = const.tile([P, P], f16, name="identh")
        nc.vector.tensor_copy(ident, identr)
        ones_raw = const.tile([1, P], f32)
        nc.vector.memset(ones_raw, 1.0)
        ones1 = const.tile([1, P], f16, name="ones1h")
        nc.vector.tensor_copy(ones1, ones_raw)

        for _rep in range(repeat):
            emit_body(
                nc, tc, btl, bc,
                x, h, c_in, dp, rdp, kw, rkw, kdp, rkdp, h_new, c_new,
                xstage, ustage, vstage, atrans, w16s, wus, wmpool, gstage,
                cpool, bstage, tpool, psum,
                ident, ones1, bias,
            )

    nc.compile()
    return nc


def emit_body(
    nc, tc, btl, bc,
    x, h, c_in, dp, rdp, kw, rkw, kdp, rkdp, h_new, c_new,
    xstage, ustage, vstage, atrans, w16s, wus, wmpool, gstage,
    cpool, bstage, tpool, psum,
    ident, ones1, bias,
):
    # Per-(kk, bt) transposed-activation tiles [128, 128] so matmuls can begin
    # as soon as the slices they need exist (fine-grained deps).
    actT = [
        [
            atrans.tile([P, P], f16, name=f"actT_{kk}_{bt}", tag=f"aT{kk}_{bt}")
            for bt in range(btl)
        ]
        for kk in range(KT)
    ]

    # bias: one DMA for all 4096 columns, pre-divided by S, cast fp16
    braw = bstage.tile([1, NG4], f32, tag="b", name="braw")
    nc.sync.dma_start(out=braw, in_=bias.unsqueeze(0))
    bias16 = bstage.tile([1, NG4], f16, tag="bs", name="bs16")
    nc.vector.tensor_scalar_mul(bias16, braw, 1.0 / S)

    # ---- Phase 1: mask activations (cast fp16), transpose them into actT ----
    # Loads/mask/transpose batched over pairs of 128-row b-tiles.
    for grp in range(btl // 2):
        rows2 = slice(grp * 2 * P, (grp + 1) * 2 * P)
        for src, usrc, kbase in ((x, dp, 0), (h, rdp, KT // 2)):
            vt = xstage.tile([P, 2, D], f16, tag="xs")
            ut = ustage.tile([P, 2, D], f32, tag="us")
            nc.sync.dma_start(
                out=vt, in_=src[rows2, :].rearrange("(j p) c -> p j c", p=P)
            )
            nc.sync.dma_start(
                out=ut, in_=usrc[rows2, :].rearrange("(j p) c -> p j c", p=P)
            )
            vm = vstage.tile([P, 2, D], f16, tag="vm")
            nc.vector.scalar_tensor_tensor(
                vm, ut, DROPOUT, vt, op0=OP.is_ge, op1=OP.mult
            )
            for jj in range(2):
                bt = grp * 2 + jj
                pt = psum.tile([P, 8, P], f16, tag="ps", name=f"pt{bt}_{kbase}")
                for j in range(8):
                    nc.tensor.transpose(
                        pt[:, j, :], vm[:, jj, j * P : (j + 1) * P], ident
                    )
                    nc.scalar.copy(actT[kbase + j][bt], pt[:, j, :])

    # ---- Phase 2: matmul chunks + gate math, in gate order c~, i, f, o ----
    # Each 512-col chunk runs as two half-passes of 4 b-tiles (PSUM stays at 8
    # banks); weight-mask tiles are built during the first half and reused.
    # Gate math for each half-pass is EMITTED one half-pass later, so its DVE
    # work gets lower scheduler priority than the next k-loop's weight-mask
    # production (keeps the PE fed) while its PSUM banks still free in time.
    # T_all[:, b, :] holds tanh(zc), then i*tanh(zc), then c', then (o-gate)
    # h', per b-tile; h'/c' are stored to DRAM in [128,4,512] blocks.
    T_all = tpool.tile([P, btl, U], f32)
    bhalf = max(1, btl // 2)

    def emit_gates(g, ucols, bs, zp, cts):
        for b in bs:
            tsl = T_all[:, b, ucols]
            if g == 2:  # candidate: T = tanh(zc)
                nc.scalar.activation(tsl, zp[b], AF.Tanh, scale=S)
            elif g == 0:  # input gate: T = sig(zi) * T
                sg = gstage.tile([P, NW], f32, tag="g", name=f"sg{b}")
                nc.scalar.activation(sg, zp[b], AF.Sigmoid, scale=S)
                nc.vector.tensor_tensor(tsl, sg, tsl, OP.mult)
            elif g == 1:  # forget gate: c' = sig(zf)*c + T
                sg = gstage.tile([P, NW], f32, tag="g", name=f"sg{b}")
                nc.scalar.activation(sg, zp[b], AF.Sigmoid, scale=S)
                nc.vector.tensor_tensor(sg, sg, cts[b], OP.mult)
                nc.vector.tensor_tensor(tsl, tsl, sg, OP.add)
            else:  # output gate: h' = sig(zo) * tanh(c'), written back to T_all
                sg = gstage.tile([P, NW], f32, tag="g", name=f"sg{b}")
                nc.scalar.activation(sg, zp[b], AF.Sigmoid, scale=S)
                tct = gstage.tile([P, NW], f32, tag="g", name=f"tct{b}")
                nc.scalar.activation(tct, tsl, AF.Tanh)
                nc.vector.tensor_tensor(tsl, sg, tct, OP.mult)
        if g in (1, 3):
            # batched store of this half-pass's 4 b-tiles from T_all
            dst = c_new if g == 1 else h_new
            b0 = bs[0]
            rows4 = slice(b0 * P, (b0 + len(bs)) * P)
            nc.scalar.dma_start(
                out=dst[rows4, ucols].rearrange("(j p) c -> p j c", p=P),
                in_=T_all[:, b0 : b0 + len(bs), ucols],
            )

    gate_order = [(2, 0), (2, 1), (0, 0), (0, 1), (1, 0), (1, 1), (3, 0), (3, 1)]
    pending = None
    for ci, (g, hf) in enumerate(gate_order):
        col0 = g * U + hf * NW
        ucols = slice(hf * NW, (hf + 1) * NW)
        wm = [None] * KT
        for bh in range(0, btl, bhalf):
            bs = range(bh, min(bh + bhalf, btl))
            zp = {
                b: psum.tile([P, NW], f32, tag="ps", name=f"z{ci}_{b}") for b in bs
            }
            # prefetch the cell state this half-pass's forget gate will need
            cts = {}
            if g == 1:
                for b in bs:
                    ct = cpool.tile([P, NW], f32, tag="ct", name=f"ct{ci}_{b}")
                    nc.scalar.dma_start(
                        out=ct, in_=c_in[b * P : (b + 1) * P, ucols]
                    )
                    cts[b] = ct
            for b in bs:
                nc.tensor.matmul(
                    zp[b],
                    lhsT=ones1,
                    rhs=bias16[:, col0 : col0 + NW],
                    start=True,
                    stop=False,
                )
            if bh == 0:
                # weight loads batched 4 k-tiles per DMA: [128, 4, 512]
                for half, (wsrc, usrc) in enumerate(((kw, kdp), (rkw, rkdp))):
                    for q in range(2):
                        r0 = q * 4 * P
                        wt = w16s.tile([P, 4, NW], f16, tag="w16")
                        uw = wus.tile([P, 4, NW], f32, tag="wu")
                        nc.sync.dma_start(
                            out=wt,
                            in_=wsrc[r0 : r0 + 4 * P, col0 : col0 + NW].rearrange(
                                "(j p) c -> p j c", p=P
                            ),
                        )
                        nc.sync.dma_start(
                            out=uw,
                            in_=usrc[r0 : r0 + 4 * P, col0 : col0 + NW].rearrange(
                                "(j p) c -> p j c", p=P
                            ),
                        )
                        for j in range(4):
                            kk = half * 8 + q * 4 + j
                            wmt = wmpool.tile(
                                [P, NW], f16, tag="wm", name=f"wm{ci}_{kk}"
                            )
                            nc.vector.scalar_tensor_tensor(
                                wmt,
                                uw[:, j, :],
                                KERNEL_DROPOUT,
                                wt[:, j, :],
                                op0=OP.is_ge,
                                op1=OP.mult,
                            )
                            wm[kk] = wmt
            for kk in range(KT):
                for b in bs:
                    nc.tensor.matmul(
                        zp[b],
                        lhsT=actT[kk][b],
                        rhs=wm[kk],
                        start=False,
                        stop=(kk == KT - 1),
                    )
            if pending is not None:
                emit_gates(*pending)
            pending = (g, ucols, list(bs), zp, cts)
    emit_gates(*pending)


_NC_CACHE: dict[tuple, object] = {}


def get_nc(bc: int = BC, repeat: int = 1):
    key = (bc, repeat)
    if key not in _NC_CACHE:
        _NC_CACHE[key] = build_nc(bc, repeat)
    return _NC_CACHE[key]


def make_in_maps(x, h, c, kernel, recurrent_kernel, bias, dp_u, rec_dp_u, k_dp_u, rk_dp_u):
    def f32a(a):
        return np.ascontiguousarray(np.asarray(a, dtype=np.float32))

    def f16a(a):
        # fp16 staging of value tensors: (u>=rate)*fp16(v) == fp16((u>=rate)*v)
        # since the mask is exactly 0 or 1, so this matches on-device rounding
        # bit for bit while halving DMA bytes.
        return np.ascontiguousarray(np.asarray(a, dtype=np.float32).astype(np.float16))

    kernel16 = f16a(kernel)
    rkernel16 = f16a(recurrent_kernel)
    bias = f32a(bias)
    k_dp_u = f32a(k_dp_u)
    rk_dp_u = f32a(rk_dp_u)
    x16, h16 = f16a(x), f16a(h)
    c, dp_u, rec_dp_u = f32a(c), f32a(dp_u), f32a(rec_dp_u)

    in_maps = []
    for ci in range(N_CORES):
        sl = slice(ci * BC, (ci + 1) * BC)
        in_maps.append(
            {
                "x": np.ascontiguousarray(x16[sl]),
                "h": np.ascontiguousarray(h16[sl]),
                "c": np.ascontiguousarray(c[sl]),
                "dp_u": np.ascontiguousarray(dp_u[sl]),
                "rec_dp_u": np.ascontiguousarray(rec_dp_u[sl]),
                "kern": kernel16,
                "rkern": rkernel16,
                "k_dp_u": k_dp_u,
                "rk_dp_u": rk_dp_u,
                "bias": bias,
            }
        )
    return in_maps


def kernel(x, h, c, kernel, recurrent_kernel, bias, dp_u, rec_dp_u, k_dp_u, rk_dp_u):
    nc = get_nc()
    in_maps = make_in_maps(
        x, h, c, kernel, recurrent_kernel, bias, dp_u, rec_dp_u, k_dp_u, rk_dp_u
    )
    res = run_bass_kernel_spmd(nc, in_maps, core_ids=list(range(N_CORES)))
    h_new = np.concatenate([res.results[ci]["h_new"] for ci in range(N_CORES)], axis=0)
    c_new = np.concatenate([res.results[ci]["c_new"] for ci in range(N_CORES)], axis=0)
    return h_new, c_new
